# revision 14
# baseline (speedup 1.0000x reference)
"""Trainium2 Bass kernel for ContextEncoderModel (siamese LSTM encoder + MLP).

Reference computation (B=256, T=128, E=300, H=512, D=1024):
  lh = LSTM_left (left_embed,  left_lengths)  -> h at t=len-1   [B, H]
  rh = LSTM_right(right_embed, right_lengths) -> h at t=len-1   [B, H]
  x  = relu(concat([lh, rh]) @ trans_w)                          [B, D]
  x  = relu(x @ hidden_ws[0]); x = relu(x @ hidden_ws[1])        [B, D]

Sharding (8 cores): core i handles side s=i//4 (0=left, 1=right) and batch
shard q=i%4 (rows q*64:(q+1)*64).  LSTM state is TRANSPOSED on device:
h^T/c^T live as [128 partitions (h-dim chunk), pair, 64 batch] tiles so the
per-step recurrent matmul z^T[m, b] += W[k, m].T @ h^T[k, b] keeps the
weights stationary (bf16 -> FWL fast weight load) with no per-step
transposes.

Ragged lengths are handled by RIGHT-ALIGNING each sequence on the host
(shift so it ends at t=T-1) and gating the bias ones-row to 0 in the padded
prefix: with zero input and zero bias the LSTM state stays exactly (0, 0),
so h at t=T-1 equals the reference's h at t=len-1.  No masks, no predicated
copies.

PSUM layout: four 4-KB super-tiles (2 banks each), phase-staggered by one
step, each covering 4 steps for one gate family x one h-chunk pair:
  A_p = [128, s(2), {i,f}, 4*64]   opens at t % 4 == p      (p = 0, 1)
  B_p = [128, s(2), {j,o}, 4*64]   opens at t % 4 == 2 + p
Batched activations read across both banks of a super-tile with one
instruction (sigmoid over i&f: FD=256), quartering ACT instruction count.
The x-part (x_t @ W_x + bias, h-independent) is emitted as N<=256 bursts
when each group opens; since exactly one group opens per step, every step's
gate-chain stall has a ~1.3us x-burst queued behind it on the PE, keeping
the PE dense (and HAM-warm).  Per step the h-part runs s01-gate matmuls
first, then s23, so the pair-01 gate chain overlaps the pair-23 matmuls.

The MLP head splits trans_w into the side's 512 rows; each core computes a
partial [D, 64] product; a pairwise AllReduce (cores q, 4+q share a batch
shard) sums them and both cores run the tiny 2-layer tail.  Host reads
cores 0-3.
"""

import numpy as np
import ml_dtypes

import concourse.mybir as mybir
import concourse.tile as tile
from concourse import bacc
from concourse.bass_utils import run_bass_kernel_spmd

BF16 = mybir.dt.bfloat16
F32 = mybir.dt.float32
AF = mybir.ActivationFunctionType

B, T, E, H, D = 256, 128, 300, 512, 1024
NCORES = 8
BC = B // 4          # 64 batch rows per core (4 shards x 2 sides)
P = 128
G = 4 * H            # 2048 gate pre-activations
KH = H // P          # 4 h-dim chunks
KE_FULL = 2          # full 128-row chunks of E
E2 = E - KE_FULL * P  # 44 leftover rows
MG = G // P          # 16 gate-dim chunks
MD = D // P          # 8 D chunks
FORGET_BIAS = 1.0
SG = 4               # steps per psum group
XC = 32              # steps per xt DMA chunk

# gate m-chunk map: i: m=0..3, j: 4..7, f: 8..11, o: 12..15 (index = h-chunk)
_GBASE = {("A", 0): 0, ("A", 1): 8, ("B", 0): 4, ("B", 1): 12}


def _group_starts(offset: int, t_steps: int):
    """Group (t0, tlen) list for a phase offset: [0,off), then 4-step runs."""
    out = []
    if offset > 0:
        out.append((0, offset))
    t0 = offset
    while t0 < t_steps:
        out.append((t0, min(SG, t_steps - t0)))
        t0 += SG
    return out


def _build(t_steps: int = T):
    nc = bacc.Bacc(
        "TRN2", target_bir_lowering=False, debug=False, num_devices=NCORES
    )
    assert t_steps % SG == 0 and t_steps >= 8
    TB = t_steps * BC

    def din(name, shape):
        return nc.dram_tensor(name, shape, BF16, kind="ExternalInput").ap()

    xt0 = din("xt0", [P, TB])
    xt1 = din("xt1", [P, TB])
    xt2 = din("xt2", [E2 + 1, TB])          # 44 embed rows + gated-ones row
    wh = din("wh", [P, KH * G])             # wh[p, k*G+c] = W[300+k*128+p, c]
    wx01 = din("wx01", [P, 2 * G])
    wx2 = din("wx2", [E2 + 1, G])           # 44 rows + adjusted-bias row
    twt = din("twt", [P, KH * D])           # side's 512 rows of trans_w
    hww = din("hww", [P, 2 * MD * D])       # both hidden layers
    out_t = nc.dram_tensor("out_t", [P, MD * BC], F32, kind="ExternalOutput").ap()

    # phase offsets: one group opens per step
    phase = {("A", 0): 0, ("A", 1): 1, ("B", 0): 2, ("B", 1): 3}
    openings = {}  # t -> list of (which, p, t0, tlen)
    for (w, p), off in phase.items():
        for t0, tlen in _group_starts(off, t_steps):
            openings.setdefault(t0, []).append((w, p, t0, tlen))

    with tile.TileContext(nc) as tc:
        with (
            tc.tile_pool(name="wpool", bufs=1) as wp,
            tc.tile_pool(name="state", bufs=1) as st,
            tc.tile_pool(name="gates", bufs=4) as gp,
            tc.tile_pool(name="psum", bufs=1, space="PSUM") as psum,
            tc.tile_pool(name="dram", bufs=2, space="DRAM") as dp,
        ):
            # ---- resident weights / inputs ----
            nxc = max(1, t_steps // XC)
            xcs = min(XC, t_steps)
            wh_sb_k = [
                wp.tile([P, G], BF16, name=f"whk{k}") for k in range(KH)
            ]
            wx01_sb = wp.tile([P, 2 * G], BF16)
            wx2_sb = wp.tile([E2 + 1, G], BF16)
            twt_sb = wp.tile([P, KH * D], BF16)
            hww_sb = wp.tile([P, 2 * MD * D], BF16)
            xt0_c = []
            xt1_c = []
            xt2_c = []
            nc.sync.dma_start(wx01_sb[:], wx01[:])
            nc.sync.dma_start(wx2_sb[:], wx2[:])
            for ci in range(nxc):
                csl = slice(ci * xcs * BC, (ci + 1) * xcs * BC)
                x0 = wp.tile([P, xcs * BC], BF16, name=f"xt0c{ci}")
                x1 = wp.tile([P, xcs * BC], BF16, name=f"xt1c{ci}")
                x2 = wp.tile([E2 + 1, xcs * BC], BF16, name=f"xt2c{ci}")
                nc.sync.dma_start(x0[:], xt0[:, csl])
                nc.sync.dma_start(x1[:], xt1[:, csl])
                nc.sync.dma_start(x2[:], xt2[:, csl])
                if ci == 0:
                    # wh after the first xt chunk: the t=0 x-bursts unblock
                    # sooner; the h-waves need wh only ~5us later
                    for k in range(KH):
                        nc.sync.dma_start(wh_sb_k[k][:], wh[:, k * G:(k + 1) * G])
                xt0_c.append(x0)
                xt1_c.append(x1)
                xt2_c.append(x2)
            nc.sync.dma_start(twt_sb[:], twt[:])
            nc.sync.dma_start(hww_sb[:], hww[:])

            # ---- persistent state (per h-chunk pair) ----
            h_t = []
            c_t = []
            for p in range(2):
                hp = gp.tile([P, 2, BC], BF16, name=f"h{p}", tag=f"h{p}")
                cp = gp.tile([P, 2, BC], F32, name=f"c{p}", tag=f"c{p}")
                nc.vector.memset(hp[:], 0.0)
                nc.vector.memset(cp[:], 0.0)
                h_t.append(hp)
                c_t.append(cp)

            groups = {}   # (which, p) -> [tile, t0, tlen, h_mms_left]
            pending = {}  # opened early (emission) but active from next step

            def xpart(ps, w, p, t0, tlen):
                first = True
                for si in range(2):
                    for g in range(2):
                        m = _GBASE[(w, g)] + 2 * p + si
                        msl = slice(m * P, (m + 1) * P)
                        m1sl = slice(G + m * P, G + (m + 1) * P)
                        done = 0
                        while done < tlen:
                            ci, o = divmod((t0 + done) * BC, xcs * BC)
                            seg = min(tlen - done, xcs - (t0 + done) % xcs)
                            rsl = slice(o, o + seg * BC)
                            osl = ps[:, si, g, done * BC:(done + seg) * BC]
                            nc.tensor.matmul(
                                osl, lhsT=wx01_sb[:, msl],
                                rhs=xt0_c[ci][:, rsl],
                                start=first, stop=False, skip_group_check=True,
                            )
                            first = False
                            nc.tensor.matmul(
                                osl, lhsT=wx01_sb[:, m1sl],
                                rhs=xt1_c[ci][:, rsl],
                                start=False, stop=False, skip_group_check=True,
                            )
                            nc.tensor.matmul(
                                osl, lhsT=wx2_sb[:, msl],
                                rhs=xt2_c[ci][:, rsl],
                                start=False, stop=False, skip_group_check=True,
                            )
                            done += seg
                    first = True  # next si = next bank: fresh has_written clear

            def open_group(w, p, t0, tlen, defer=False):
                ps = psum.tile([P, 2, 2, SG * BC], F32, tag=f"ps{w}{p}",
                               name=f"ps{w}{p}_{t0}")
                xpart(ps, w, p, t0, tlen)
                g = [ps, t0, tlen, 16 * tlen]
                if defer:
                    pending[(w, p)] = g
                else:
                    groups[(w, p)] = g

            # ---- recurrence ----
            for t in range(t_steps):
                # 1) open groups due this step.  At t=0 the x-bursts go in
                # front (banks are fresh, h-waves need the big wh DMA);
                # afterwards they are emitted after the waves so the PE FIFO
                # reaches them during the gate-chain stall (their WAR dep on
                # the dying group's last sigma read resolves mid-stall).
                groups.update(pending)
                pending.clear()
                if t == 0:
                    for (w, p, t0, tlen) in openings.get(t, []):
                        open_group(w, p, t0, tlen)

                # 2) h-part waves: ALL of pair-01's gates first (its banks
                # stop ~1.8us into the step so its chain overlaps pair-23's
                # matmuls), then pair-23
                for pg in range(2):
                    for k in range(KH):
                        rhs = h_t[k // 2][:, k % 2, :]
                        for w in ("A", "B"):
                            gr = groups[(w, pg)]
                            ps, t0, tlen, left = gr
                            ca = t - t0
                            for g in range(2):
                                for si in range(2):
                                    left -= 1
                                    m = _GBASE[(w, g)] + 2 * pg + si
                                    nc.tensor.matmul(
                                        ps[:, si, g, ca * BC:(ca + 1) * BC],
                                        lhsT=wh_sb_k[k][:, m * P:(m + 1) * P],
                                        rhs=rhs,
                                        start=False, stop=(left == 0),
                                        skip_group_check=True,
                                    )
                            gr[3] = left

                # 2b) x-bursts for groups opening next step
                if t + 1 < t_steps:
                    for (w, p, t0, tlen) in openings.get(t + 1, []):
                        open_group(w, p, t0, tlen, defer=True)

                # 3) gate chains, pair-01 then pair-23
                h_new = []
                c_new = []
                for p in range(2):
                    psA, tA, _, _ = groups[("A", p)]
                    psB, tB, _, _ = groups[("B", p)]
                    ca, cb = t - tA, t - tB
                    sif = gp.tile([P, 2, 2, BC], F32, name=f"sif{p}",
                                  tag=f"sif{p}")
                    tj = gp.tile([P, 2, BC], F32, name=f"tj{p}", tag=f"tj{p}")
                    so = gp.tile([P, 2, BC], F32, name=f"so{p}", tag=f"so{p}")
                    nc.scalar.activation(
                        sif[:], psA[:, :, :, ca * BC:(ca + 1) * BC], AF.Sigmoid
                    )
                    nc.scalar.activation(
                        tj[:], psB[:, :, 0, cb * BC:(cb + 1) * BC], AF.Tanh
                    )
                    nc.scalar.activation(
                        so[:], psB[:, :, 1, cb * BC:(cb + 1) * BC], AF.Sigmoid
                    )
                    ta = gp.tile([P, 2, BC], F32, name=f"ta{p}", tag=f"ta{p}")
                    tb = gp.tile([P, 2, BC], F32, name=f"tb{p}", tag=f"tb{p}")
                    cn = gp.tile([P, 2, BC], F32, name=f"cn{p}", tag=f"c{p}")
                    th = gp.tile([P, 2, BC], F32, name=f"th{p}", tag=f"th{p}")
                    hn = gp.tile([P, 2, BC], BF16, name=f"hn{p}", tag=f"h{p}")
                    nc.gpsimd.tensor_mul(ta[:], c_t[p][:], sif[:, :, 1, :])
                    nc.vector.tensor_mul(tb[:], sif[:, :, 0, :], tj[:])
                    nc.vector.tensor_add(cn[:], ta[:], tb[:])
                    nc.scalar.activation(th[:], cn[:], AF.Tanh)
                    nc.vector.tensor_mul(hn[:], th[:], so[:])
                    h_new.append(hn)
                    c_new.append(cn)
                h_t = h_new
                c_t = c_new

            # ---- MLP head ----
            # partial = (side rows of trans_w).T @ h^T  -> [D, 64] transposed
            p_sb = st.tile([P, MD * BC], BF16)
            for m in range(MD):
                ps = psum.tile([P, BC], F32, tag=f"ps{'AB'[m % 2]}0",
                               name=f"hd{m}")
                for k in range(KH):
                    nc.tensor.matmul(
                        ps[:],
                        lhsT=twt_sb[:, k * D + m * P:k * D + (m + 1) * P],
                        rhs=h_t[k // 2][:, k % 2, :],
                        start=(k == 0), stop=(k == KH - 1),
                    )
                nc.vector.tensor_copy(p_sb[:, m * BC:(m + 1) * BC], ps[:])

            cin = dp.tile([P, MD * BC], BF16)
            cout = dp.tile([P, MD * BC], BF16)
            nc.sync.dma_start(cin[:], p_sb[:])
            nc.gpsimd.collective_compute(
                "AllReduce",
                mybir.AluOpType.add,
                replica_groups=[[0, 1], [2, 3], [4, 5], [6, 7]],
                ins=[cin.opt()],
                outs=[cout.opt()],
            )
            x1pre = st.tile([P, MD * BC], BF16)
            nc.sync.dma_start(x1pre[:], cout[:])
            xcur = st.tile([P, MD * BC], BF16)
            nc.scalar.activation(xcur[:], x1pre[:], AF.Relu)

            for layer in range(2):
                nxt = st.tile([P, MD * BC], BF16, tag=f"x{layer + 1}")
                out_f32 = None
                if layer == 1:
                    out_f32 = st.tile([P, MD * BC], F32, name="out_f32")
                for m in range(MD):
                    ps = psum.tile([P, BC], F32, tag=f"ps{'AB'[m % 2]}1",
                                   name=f"ly{layer}_{m}")
                    for k in range(MD):
                        off = (layer * MD + k) * D
                        nc.tensor.matmul(
                            ps[:],
                            lhsT=hww_sb[:, off + m * P:off + (m + 1) * P],
                            rhs=xcur[:, k * BC:(k + 1) * BC],
                            start=(k == 0), stop=(k == MD - 1),
                        )
                    if layer == 0:
                        nc.scalar.activation(
                            nxt[:, m * BC:(m + 1) * BC], ps[:], AF.Relu
                        )
                    else:
                        nc.scalar.activation(
                            out_f32[:, m * BC:(m + 1) * BC], ps[:], AF.Relu
                        )
                xcur = nxt
            nc.sync.dma_start(out_t[:], out_f32[:])

    nc.compile()
    return nc


_BUILD_CACHE: dict = {}


def _get_nc(t_steps: int = T):
    if t_steps not in _BUILD_CACHE:
        _BUILD_CACHE[t_steps] = _build(t_steps)
    return _BUILD_CACHE[t_steps]


def _core_inputs(embed, lengths, Wf, bf, trans_w, hidden_ws, side, t_steps):
    """Build the per-core input dict. embed [BC,T,E] f32, lengths [BC] i32."""
    bf16 = ml_dtypes.bfloat16
    TB = t_steps * BC

    # right-align sequences; gate the bias ones-row off in the padding
    em = np.asarray(embed[:, :t_steps, :], dtype=np.float32)
    xs = np.zeros_like(em)
    ones = np.zeros((BC, t_steps), dtype=np.float32)
    ln = np.minimum(np.asarray(lengths, dtype=np.int64), t_steps)
    for bi in range(BC):
        L = int(ln[bi])
        xs[bi, t_steps - L:] = em[bi, :L]
        ones[bi, t_steps - L:] = 1.0

    # x transposed: xt[e, t, b]
    xt = np.ascontiguousarray(xs.transpose(2, 1, 0)).astype(bf16)
    xt0 = xt[0:P].reshape(P, TB)
    xt1 = xt[P:2 * P].reshape(P, TB)
    xt2 = np.empty((E2 + 1, TB), dtype=bf16)
    xt2[:E2] = xt[2 * P:E].reshape(E2, TB)
    xt2[E2] = np.ascontiguousarray(ones.T).reshape(TB).astype(bf16)

    Wb = Wf.astype(np.float32)
    wh = np.ascontiguousarray(
        Wb[E:].reshape(KH, P, G).transpose(1, 0, 2).reshape(P, KH * G)
    ).astype(bf16)
    wx01 = np.ascontiguousarray(
        Wb[0:2 * P].reshape(2, P, G).transpose(1, 0, 2).reshape(P, 2 * G)
    ).astype(bf16)
    badj = bf.astype(np.float32).copy()
    badj[2 * H:3 * H] += FORGET_BIAS
    wx2 = np.empty((E2 + 1, G), dtype=bf16)
    wx2[:E2] = Wb[2 * P:E].astype(bf16)
    wx2[E2] = badj.astype(bf16)

    tw = trans_w[side * H:(side + 1) * H].astype(np.float32)
    twt = np.ascontiguousarray(
        tw.reshape(KH, P, D).transpose(1, 0, 2).reshape(P, KH * D)
    ).astype(bf16)
    hww = np.ascontiguousarray(
        hidden_ws.astype(np.float32).reshape(2, MD, P, D)
        .transpose(2, 0, 1, 3).reshape(P, 2 * MD * D)
    ).astype(bf16)

    return dict(xt0=xt0, xt1=xt1, xt2=xt2, wh=wh, wx01=wx01, wx2=wx2,
                twt=twt, hww=hww)


def prepare_in_maps(left_embed, right_embed, left_lengths, right_lengths,
                    W_left, b_left, W_right, b_right, trans_w, hidden_ws,
                    t_steps=T):
    in_maps = []
    for core in range(NCORES):
        # pair adjacent cores (2q, 2q+1) = (left, right) for one batch shard
        # so the head AllReduce stays within a core pair
        q, side = divmod(core, 2)
        rows = slice(q * BC, (q + 1) * BC)
        if side == 0:
            emb, ln, Wf, bf = left_embed[rows], left_lengths[rows], W_left, b_left
        else:
            emb, ln, Wf, bf = right_embed[rows], right_lengths[rows], W_right, b_right
        in_maps.append(
            _core_inputs(np.asarray(emb), np.asarray(ln), np.asarray(Wf),
                         np.asarray(bf), np.asarray(trans_w),
                         np.asarray(hidden_ws), side, t_steps)
        )
    return in_maps


def _assemble(results):
    out = np.empty((B, D), dtype=np.float32)
    for q in range(4):
        # out_t [P, MD, BC] with out_t[p, m, b] = y[q*64+b, m*128+p]
        ot = results[2 * q]["out_t"].reshape(P, MD, BC)
        out[q * BC:(q + 1) * BC] = ot.transpose(2, 1, 0).reshape(BC, D)
    return out


def kernel(left_embed, right_embed, left_lengths, right_lengths,
           W_left, b_left, W_right, b_right, trans_w, hidden_ws):
    nc = _get_nc(T)
    in_maps = prepare_in_maps(
        left_embed, right_embed, left_lengths, right_lengths,
        W_left, b_left, W_right, b_right, trans_w, hidden_ws, T
    )
    r = run_bass_kernel_spmd(nc, in_maps, list(range(NCORES)))
    return _assemble(r.results)


def run_traced(inputs, t_steps=T, **trace_kwargs):
    """test.py helper: run with NTFF tracing, return (output, BassKernelResults)."""
    nc = _get_nc(t_steps)
    in_maps = prepare_in_maps(t_steps=t_steps, **inputs)
    r = run_bass_kernel_spmd(
        nc, in_maps, list(range(NCORES)), trace=True, **trace_kwargs
    )
    return _assemble(r.results), r


# revision 21
# speedup vs baseline: 1.1185x; 1.1185x over previous
"""Trainium2 Bass kernel for ContextEncoderModel (siamese LSTM encoder + MLP).

Reference computation (B=256, T=128, E=300, H=512, D=1024):
  lh = LSTM_left (left_embed,  left_lengths)  -> h at t=len-1   [B, H]
  rh = LSTM_right(right_embed, right_lengths) -> h at t=len-1   [B, H]
  x  = relu(concat([lh, rh]) @ trans_w)                          [B, D]
  x  = relu(x @ hidden_ws[0]); x = relu(x @ hidden_ws[1])        [B, D]

Sharding (8 cores): core i handles side s=i//4 (0=left, 1=right) and batch
shard q=i%4 (rows q*64:(q+1)*64).  LSTM state is TRANSPOSED on device:
h^T/c^T live as [128 partitions (h-dim chunk), pair, 64 batch] tiles so the
per-step recurrent matmul z^T[m, b] += W[k, m].T @ h^T[k, b] keeps the
weights stationary (bf16 -> FWL fast weight load) with no per-step
transposes.

Ragged lengths are handled by RIGHT-ALIGNING each sequence on the host
(shift so it ends at t=T-1) and gating the bias ones-row to 0 in the padded
prefix: with zero input and zero bias the LSTM state stays exactly (0, 0),
so h at t=T-1 equals the reference's h at t=len-1.  No masks, no predicated
copies.

PSUM layout: four super-tiles phase-staggered by one step, each covering 4
steps for one gate family x one h-chunk pair:
  A_p = [128, s(2), {i,f,o}, 4*64]  (3 banks)  opens at t % 4 == p
  B_p = [128, s(2), {j},     4*64]  (1 bank)   opens at t % 4 == 2 + p
Batched activations read across the banks of a super-tile with one
instruction (sigmoid over i,f,o: FD=384), cutting ACT instruction count
from 16 to 6 per step.
The x-part (x_t @ W_x + bias, h-independent) is emitted as N<=256 bursts
when each group opens; since exactly one group opens per step, every step's
gate-chain stall has a ~1.3us x-burst queued behind it on the PE, keeping
the PE dense (and HAM-warm).  Per step the h-part runs s01-gate matmuls
first, then s23, so the pair-01 gate chain overlaps the pair-23 matmuls.

The MLP head splits trans_w into the side's 512 rows; each core computes a
partial [D, 64] product; a pairwise AllReduce (cores q, 4+q share a batch
shard) sums them and both cores run the tiny 2-layer tail.  Host reads
cores 0-3.
"""

import numpy as np
import ml_dtypes

import concourse.mybir as mybir
import concourse.tile as tile
from concourse import bacc
from concourse.bass_utils import run_bass_kernel_spmd

BF16 = mybir.dt.bfloat16
F32 = mybir.dt.float32
AF = mybir.ActivationFunctionType

B, T, E, H, D = 256, 128, 300, 512, 1024
NCORES = 8
BC = B // 4          # 64 batch rows per core (4 shards x 2 sides)
P = 128
G = 4 * H            # 2048 gate pre-activations
KH = H // P          # 4 h-dim chunks
KE_FULL = 2          # full 128-row chunks of E
E2 = E - KE_FULL * P  # 44 leftover rows
MG = G // P          # 16 gate-dim chunks
MD = D // P          # 8 D chunks
FORGET_BIAS = 1.0
SG = 4               # steps per psum group
XC = 32              # steps per xt DMA chunk

# gate m-chunk map: i: m=0..3, j: 4..7, f: 8..11, o: 12..15 (index = h-chunk)
# A supers hold the sigmoid family {i, f, o} (3 banks), B supers hold {j}
_GBASE = {("A", 0): 0, ("A", 1): 8, ("A", 2): 12, ("B", 0): 4}
_NGATES = {"A": 3, "B": 1}


def _group_starts(offset: int, t_steps: int):
    """Group (t0, tlen) list for a phase offset: [0,off), then 4-step runs."""
    out = []
    if offset > 0:
        out.append((0, offset))
    t0 = offset
    while t0 < t_steps:
        out.append((t0, min(SG, t_steps - t0)))
        t0 += SG
    return out


def _build(t_steps: int = T):
    nc = bacc.Bacc(
        "TRN2", target_bir_lowering=False, debug=False, num_devices=NCORES
    )
    assert t_steps % SG == 0 and t_steps >= 8
    TB = t_steps * BC

    def din(name, shape):
        return nc.dram_tensor(name, shape, BF16, kind="ExternalInput").ap()

    xt0 = din("xt0", [P, TB])
    xt1 = din("xt1", [P, TB])
    xt2 = din("xt2", [E2 + 1, TB])          # 44 embed rows + gated-ones row
    wh = din("wh", [P, KH * G])             # wh[p, k*G+c] = W[300+k*128+p, c]
    wx01 = din("wx01", [P, 2 * G])
    wx2 = din("wx2", [E2 + 1, G])           # 44 rows + adjusted-bias row
    twt = din("twt", [P, KH * D])           # side's 512 rows of trans_w
    hww = din("hww", [P, 2 * MD * D])       # both hidden layers
    out_t = nc.dram_tensor("out_t", [P, MD * BC], F32, kind="ExternalOutput").ap()

    # phase offsets: one group opens per step
    phase = {("A", 0): 0, ("A", 1): 1, ("B", 0): 2, ("B", 1): 3}
    openings = {}  # t -> list of (which, p, t0, tlen)
    for (w, p), off in phase.items():
        for t0, tlen in _group_starts(off, t_steps):
            openings.setdefault(t0, []).append((w, p, t0, tlen))

    with tile.TileContext(nc) as tc:
        with (
            tc.tile_pool(name="wpool", bufs=1) as wp,
            tc.tile_pool(name="state", bufs=1) as st,
            tc.tile_pool(name="gates", bufs=4) as gp,
            tc.tile_pool(name="psum", bufs=1, space="PSUM") as psum,
            tc.tile_pool(name="dram", bufs=2, space="DRAM") as dp,
        ):
            # ---- resident weights / inputs ----
            nxc = max(1, t_steps // XC)
            xcs = min(XC, t_steps)
            wh_sb_k = [
                wp.tile([P, G], BF16, name=f"whk{k}") for k in range(KH)
            ]
            wx01_sb = wp.tile([P, 2 * G], BF16)
            wx2_sb = wp.tile([E2 + 1, G], BF16)
            twt_sb = wp.tile([P, KH * D], BF16)
            hww_sb = wp.tile([P, 2 * MD * D], BF16)
            xt0_c = []
            xt1_c = []
            xt2_c = []
            nc.sync.dma_start(wx01_sb[:], wx01[:])
            nc.sync.dma_start(wx2_sb[:], wx2[:])
            for ci in range(nxc):
                csl = slice(ci * xcs * BC, (ci + 1) * xcs * BC)
                x0 = wp.tile([P, xcs * BC], BF16, name=f"xt0c{ci}")
                x1 = wp.tile([P, xcs * BC], BF16, name=f"xt1c{ci}")
                x2 = wp.tile([E2 + 1, xcs * BC], BF16, name=f"xt2c{ci}")
                nc.sync.dma_start(x0[:], xt0[:, csl])
                nc.sync.dma_start(x1[:], xt1[:, csl])
                nc.sync.dma_start(x2[:], xt2[:, csl])
                if ci == 0:
                    # wh after the first xt chunk: the t=0 x-bursts unblock
                    # sooner; the h-waves need wh only ~5us later
                    for k in range(KH):
                        nc.sync.dma_start(wh_sb_k[k][:], wh[:, k * G:(k + 1) * G])
                xt0_c.append(x0)
                xt1_c.append(x1)
                xt2_c.append(x2)
            nc.sync.dma_start(twt_sb[:], twt[:])
            nc.sync.dma_start(hww_sb[:], hww[:])

            # ---- persistent state (per h-chunk pair) ----
            h_t = []
            c_t = []
            for p in range(2):
                hp = gp.tile([P, 2, BC], BF16, name=f"h{p}", tag=f"h{p}")
                cp = gp.tile([P, 2, BC], F32, name=f"c{p}", tag=f"c{p}")
                nc.vector.memset(hp[:], 0.0)
                nc.vector.memset(cp[:], 0.0)
                h_t.append(hp)
                c_t.append(cp)

            groups = {}   # (which, p) -> [tile, t0, tlen, h_mms_left]
            pending = {}  # opened early (emission) but active from next step

            def xpart(ps, w, p, t0, tlen):
                # start=True once per 2-KB bank, on the first MM into it.
                # Region r = si*ngates + g starts a new bank when r is even.
                ngates = _NGATES[w]
                for si in range(2):
                    for g in range(ngates):
                        first = (si * ngates + g) % 2 == 0
                        m = _GBASE[(w, g)] + 2 * p + si
                        msl = slice(m * P, (m + 1) * P)
                        m1sl = slice(G + m * P, G + (m + 1) * P)
                        done = 0
                        while done < tlen:
                            ci, o = divmod((t0 + done) * BC, xcs * BC)
                            seg = min(tlen - done, xcs - (t0 + done) % xcs)
                            rsl = slice(o, o + seg * BC)
                            osl = ps[:, si, g, done * BC:(done + seg) * BC]
                            nc.tensor.matmul(
                                osl, lhsT=wx01_sb[:, msl],
                                rhs=xt0_c[ci][:, rsl],
                                start=first, stop=False, skip_group_check=True,
                            )
                            first = False
                            nc.tensor.matmul(
                                osl, lhsT=wx01_sb[:, m1sl],
                                rhs=xt1_c[ci][:, rsl],
                                start=False, stop=False, skip_group_check=True,
                            )
                            nc.tensor.matmul(
                                osl, lhsT=wx2_sb[:, msl],
                                rhs=xt2_c[ci][:, rsl],
                                start=False, stop=False, skip_group_check=True,
                            )
                            done += seg

            def open_group(w, p, t0, tlen, defer=False):
                ng = _NGATES[w]
                ps = psum.tile([P, 2, ng, SG * BC], F32, tag=f"ps{w}{p}",
                               name=f"ps{w}{p}_{t0}")
                xpart(ps, w, p, t0, tlen)
                g = [ps, t0, tlen, 8 * ng * tlen]
                if defer:
                    pending[(w, p)] = g
                else:
                    groups[(w, p)] = g

            # ---- recurrence ----
            for t in range(t_steps):
                # 1) open groups due this step.  At t=0 the x-bursts go in
                # front (banks are fresh, h-waves need the big wh DMA);
                # afterwards they are emitted after the waves so the PE FIFO
                # reaches them during the gate-chain stall (their WAR dep on
                # the dying group's last sigma read resolves mid-stall).
                groups.update(pending)
                pending.clear()
                if t == 0:
                    for (w, p, t0, tlen) in openings.get(t, []):
                        open_group(w, p, t0, tlen)

                # 2) h-part waves: ALL of pair-01's gates first (its banks
                # stop ~1.8us into the step so its chain overlaps pair-23's
                # matmuls), then pair-23
                for pg in range(2):
                    for k in range(KH):
                        rhs = h_t[k // 2][:, k % 2, :]
                        for w in ("A", "B"):
                            gr = groups[(w, pg)]
                            ps, t0, tlen, left = gr
                            ca = t - t0
                            for g in range(_NGATES[w]):
                                for si in range(2):
                                    left -= 1
                                    m = _GBASE[(w, g)] + 2 * pg + si
                                    nc.tensor.matmul(
                                        ps[:, si, g, ca * BC:(ca + 1) * BC],
                                        lhsT=wh_sb_k[k][:, m * P:(m + 1) * P],
                                        rhs=rhs,
                                        start=False, stop=(left == 0),
                                        skip_group_check=True,
                                    )
                            gr[3] = left

                # 2b) x-bursts for groups opening next step
                if t + 1 < t_steps:
                    for (w, p, t0, tlen) in openings.get(t + 1, []):
                        open_group(w, p, t0, tlen, defer=True)

                # 3) gate chains, pair-01 then pair-23
                h_new = []
                c_new = []
                for p in range(2):
                    psA, tA, _, _ = groups[("A", p)]
                    psB, tB, _, _ = groups[("B", p)]
                    ca, cb = t - tA, t - tB
                    sif = gp.tile([P, 2, 3, BC], F32, name=f"sif{p}",
                                  tag=f"sif{p}")
                    tj = gp.tile([P, 2, BC], F32, name=f"tj{p}", tag=f"tj{p}")
                    nc.scalar.activation(
                        sif[:], psA[:, :, :, ca * BC:(ca + 1) * BC], AF.Sigmoid
                    )
                    nc.scalar.activation(
                        tj[:], psB[:, :, 0, cb * BC:(cb + 1) * BC], AF.Tanh
                    )
                    ta = gp.tile([P, 2, BC], F32, name=f"ta{p}", tag=f"ta{p}")
                    tb = gp.tile([P, 2, BC], F32, name=f"tb{p}", tag=f"tb{p}")
                    cn = gp.tile([P, 2, BC], F32, name=f"cn{p}", tag=f"c{p}")
                    th = gp.tile([P, 2, BC], F32, name=f"th{p}", tag=f"th{p}")
                    hn = gp.tile([P, 2, BC], BF16, name=f"hn{p}", tag=f"h{p}")
                    nc.vector.tensor_mul(ta[:], c_t[p][:], sif[:, :, 1, :])
                    nc.vector.tensor_mul(tb[:], sif[:, :, 0, :], tj[:])
                    nc.vector.tensor_add(cn[:], ta[:], tb[:])
                    nc.scalar.activation(th[:], cn[:], AF.Tanh)
                    nc.vector.tensor_mul(hn[:], th[:], sif[:, :, 2, :])
                    h_new.append(hn)
                    c_new.append(cn)
                h_t = h_new
                c_t = c_new

            # ---- MLP head ----
            # partial = (side rows of trans_w).T @ h^T  -> [D, 64] transposed
            p_sb = st.tile([P, MD * BC], BF16)
            for m in range(MD):
                ps = psum.tile([P, BC], F32, tag=f"ps{'AB'[m % 2]}0",
                               name=f"hd{m}")
                for k in range(KH):
                    nc.tensor.matmul(
                        ps[:],
                        lhsT=twt_sb[:, k * D + m * P:k * D + (m + 1) * P],
                        rhs=h_t[k // 2][:, k % 2, :],
                        start=(k == 0), stop=(k == KH - 1),
                    )
                nc.vector.tensor_copy(p_sb[:, m * BC:(m + 1) * BC], ps[:])

            cin = dp.tile([P, MD * BC], BF16)
            cout = dp.tile([P, MD * BC], BF16)
            nc.sync.dma_start(cin[:], p_sb[:])
            nc.gpsimd.collective_compute(
                "AllReduce",
                mybir.AluOpType.add,
                replica_groups=[[0, 1], [2, 3], [4, 5], [6, 7]],
                ins=[cin.opt()],
                outs=[cout.opt()],
            )
            x1pre = st.tile([P, MD * BC], BF16)
            nc.sync.dma_start(x1pre[:], cout[:])
            xcur = st.tile([P, MD * BC], BF16)
            nc.scalar.activation(xcur[:], x1pre[:], AF.Relu)

            for layer in range(2):
                nxt = st.tile([P, MD * BC], BF16, tag=f"x{layer + 1}")
                out_f32 = None
                if layer == 1:
                    out_f32 = st.tile([P, MD * BC], F32, name="out_f32")
                for m in range(MD):
                    ps = psum.tile([P, BC], F32, tag=f"ps{'AB'[m % 2]}1",
                                   name=f"ly{layer}_{m}")
                    for k in range(MD):
                        off = (layer * MD + k) * D
                        nc.tensor.matmul(
                            ps[:],
                            lhsT=hww_sb[:, off + m * P:off + (m + 1) * P],
                            rhs=xcur[:, k * BC:(k + 1) * BC],
                            start=(k == 0), stop=(k == MD - 1),
                        )
                    if layer == 0:
                        nc.scalar.activation(
                            nxt[:, m * BC:(m + 1) * BC], ps[:], AF.Relu
                        )
                    else:
                        nc.scalar.activation(
                            out_f32[:, m * BC:(m + 1) * BC], ps[:], AF.Relu
                        )
                xcur = nxt
            nc.sync.dma_start(out_t[:], out_f32[:])

    nc.compile()
    return nc


_BUILD_CACHE: dict = {}


def _get_nc(t_steps: int = T):
    if t_steps not in _BUILD_CACHE:
        _BUILD_CACHE[t_steps] = _build(t_steps)
    return _BUILD_CACHE[t_steps]


def _core_inputs(embed, lengths, Wf, bf, trans_w, hidden_ws, side, t_steps):
    """Build the per-core input dict. embed [BC,T,E] f32, lengths [BC] i32."""
    bf16 = ml_dtypes.bfloat16
    TB = t_steps * BC

    # right-align sequences; gate the bias ones-row off in the padding
    em = np.asarray(embed[:, :t_steps, :], dtype=np.float32)
    xs = np.zeros_like(em)
    ones = np.zeros((BC, t_steps), dtype=np.float32)
    ln = np.minimum(np.asarray(lengths, dtype=np.int64), t_steps)
    for bi in range(BC):
        L = int(ln[bi])
        xs[bi, t_steps - L:] = em[bi, :L]
        ones[bi, t_steps - L:] = 1.0

    # x transposed: xt[e, t, b]
    xt = np.ascontiguousarray(xs.transpose(2, 1, 0)).astype(bf16)
    xt0 = xt[0:P].reshape(P, TB)
    xt1 = xt[P:2 * P].reshape(P, TB)
    xt2 = np.empty((E2 + 1, TB), dtype=bf16)
    xt2[:E2] = xt[2 * P:E].reshape(E2, TB)
    xt2[E2] = np.ascontiguousarray(ones.T).reshape(TB).astype(bf16)

    Wb = Wf.astype(np.float32)
    wh = np.ascontiguousarray(
        Wb[E:].reshape(KH, P, G).transpose(1, 0, 2).reshape(P, KH * G)
    ).astype(bf16)
    wx01 = np.ascontiguousarray(
        Wb[0:2 * P].reshape(2, P, G).transpose(1, 0, 2).reshape(P, 2 * G)
    ).astype(bf16)
    badj = bf.astype(np.float32).copy()
    badj[2 * H:3 * H] += FORGET_BIAS
    wx2 = np.empty((E2 + 1, G), dtype=bf16)
    wx2[:E2] = Wb[2 * P:E].astype(bf16)
    wx2[E2] = badj.astype(bf16)

    tw = trans_w[side * H:(side + 1) * H].astype(np.float32)
    twt = np.ascontiguousarray(
        tw.reshape(KH, P, D).transpose(1, 0, 2).reshape(P, KH * D)
    ).astype(bf16)
    hww = np.ascontiguousarray(
        hidden_ws.astype(np.float32).reshape(2, MD, P, D)
        .transpose(2, 0, 1, 3).reshape(P, 2 * MD * D)
    ).astype(bf16)

    return dict(xt0=xt0, xt1=xt1, xt2=xt2, wh=wh, wx01=wx01, wx2=wx2,
                twt=twt, hww=hww)


def prepare_in_maps(left_embed, right_embed, left_lengths, right_lengths,
                    W_left, b_left, W_right, b_right, trans_w, hidden_ws,
                    t_steps=T):
    in_maps = []
    for core in range(NCORES):
        # pair adjacent cores (2q, 2q+1) = (left, right) for one batch shard
        # so the head AllReduce stays within a core pair
        q, side = divmod(core, 2)
        rows = slice(q * BC, (q + 1) * BC)
        if side == 0:
            emb, ln, Wf, bf = left_embed[rows], left_lengths[rows], W_left, b_left
        else:
            emb, ln, Wf, bf = right_embed[rows], right_lengths[rows], W_right, b_right
        in_maps.append(
            _core_inputs(np.asarray(emb), np.asarray(ln), np.asarray(Wf),
                         np.asarray(bf), np.asarray(trans_w),
                         np.asarray(hidden_ws), side, t_steps)
        )
    return in_maps


def _assemble(results):
    out = np.empty((B, D), dtype=np.float32)
    for q in range(4):
        # out_t [P, MD, BC] with out_t[p, m, b] = y[q*64+b, m*128+p]
        ot = results[2 * q]["out_t"].reshape(P, MD, BC)
        out[q * BC:(q + 1) * BC] = ot.transpose(2, 1, 0).reshape(BC, D)
    return out


def kernel(left_embed, right_embed, left_lengths, right_lengths,
           W_left, b_left, W_right, b_right, trans_w, hidden_ws):
    nc = _get_nc(T)
    in_maps = prepare_in_maps(
        left_embed, right_embed, left_lengths, right_lengths,
        W_left, b_left, W_right, b_right, trans_w, hidden_ws, T
    )
    r = run_bass_kernel_spmd(nc, in_maps, list(range(NCORES)))
    return _assemble(r.results)


def run_traced(inputs, t_steps=T, **trace_kwargs):
    """test.py helper: run with NTFF tracing, return (output, BassKernelResults)."""
    nc = _get_nc(t_steps)
    in_maps = prepare_in_maps(t_steps=t_steps, **inputs)
    r = run_bass_kernel_spmd(
        nc, in_maps, list(range(NCORES)), trace=True, **trace_kwargs
    )
    return _assemble(r.results), r
